# revision 13
# baseline (speedup 1.0000x reference)
"""GCNConv (COO SpMM + feature transform) distributed over 8 NeuronCores.

out = segment_sum(x[cols] * vals, rows) @ weight

Strategy (1D row partition per the CAGNET-style hint): core k owns dest rows
[12500k, 12500(k+1)) and the edges targeting them; x and the 32x32 weight are
replicated.

The kernel is bound by GpSimd SWDGE descriptor generation (~8.6ns per
gathered row, serial on the Pool engine — measured; the cost-model's
994ns-fixed + 0.34ns/desc is wrong for the indirect/gather ucode paths, so
batching descriptors into fewer instructions does NOT help).  The design
therefore minimizes DESCRIPTOR COUNT and keeps the Pool engine saturated:
 - Per (core, class=col%NCLS): rows sharing class-cols are clustered (capped
   union-find) so edges with an identical col share ONE gather slot (the
   bval one-hot column for that slot simply has several nonzeros).  The
   class split is scale-invariant in col-collision rate (lambda=2 per
   class-col at any NCLS) but fewer edges/row/class means deeper clusters:
   NCLS=16 realizes ~50% slot dedup vs ~21% at NCLS=4.
 - Tiles: 128 slots (one indirect DMA each), <=W packed dest rows; TPS
   tiles accumulate into one PSUM bank zt[32, 512] (every column written,
   pad tiles write zeros).  W=256/TPS=2 keeps the row side (rows appear
   once per class they have edges in) from becoming the binding constraint.
 - All gather offsets live SBUF-resident (one load at startup), so the
   gathers issue back-to-back with no per-tile dependency stalls.
 - Eviction applies the weight (4 f32 matmuls) and stores a contiguous
   [128, 128] f32 block per super-block; the host un-permutes packed rows
   (np.add.at sums rows split across tiles/classes).
 - Program shape depends only on NSB -> one NEFF runs SPMD on all 8 cores.
"""

import os
import sys
import tempfile
import types

import numpy as np

# A transiently-wedged device can leave a poisoned NEFF in the shared neuron
# compile cache, making every later invocation with the same cache key crash
# (observed: NRT_EXEC_UNIT_UNRECOVERABLE on known-good programs).  Compiling
# is only a few seconds here, so use a fresh per-process cache instead.
os.environ["NEURON_COMPILE_CACHE_URL"] = tempfile.mkdtemp(prefix="neuron-cc-cache-")


def _install_ntff_hook_shim():
    """bass_utils' axon trace path imports antenv.axon_hooks, which this
    container image lacks.  Provide it (with the real ctypes-based profiler
    hook when available) so BASS_TRACE=1 in the environment doesn't crash."""
    if "antenv.axon_hooks" in sys.modules:
        return
    mod = types.ModuleType("antenv.axon_hooks")
    _h = [None]
    mod.set_axon_ntff_profile_hook = lambda h: _h.__setitem__(0, h)
    mod.get_axon_ntff_profile_hook = lambda: _h[0]
    sys.modules["antenv.axon_hooks"] = mod
    try:
        from trn_agent_boot.trn_boot import _ntff_profile_via_ctypes

        mod.set_axon_ntff_profile_hook(
            _ntff_profile_via_ctypes("/opt/axon/libaxon_pjrt.so")
        )
    except Exception:
        pass


_install_ntff_hook_shim()

import concourse.bass as bass
import concourse.mybir as mybir
import concourse.tile as tile
from concourse import bacc
from concourse.bass import IndirectOffsetOnAxis
from concourse.bass_utils import run_bass_kernel_spmd

N_NODES = 100_000
N_CORES = 8
RPC = N_NODES // N_CORES  # rows per core
F = 32
NCLS = 16                 # classes = col % 16
W = 256                   # packed output cols per tile (max rows per tile)
TPS = 2                   # tiles per super-block (2*256 = 512 PSUM cols)
P = 128                   # slots per tile

MAXR = 250                # cluster cap: rows
MAXE = 512                # cluster cap: edges

f32 = mybir.dt.float32
i32 = mybir.dt.int32

_compiled_cache = {}


class _Item:
    """A packable cluster piece: `cols` (one gather slot per unique col) and
    CSR slot->edges arrays (row = GLOBAL dest row)."""

    __slots__ = ("cols", "eoff", "erow", "eval_", "nrows")

    def __init__(self, cols, eoff, erow, eval_, nrows):
        self.cols = cols
        self.eoff = eoff      # [len(cols)+1] edge offsets per slot
        self.erow = erow      # edge rows, grouped by slot
        self.eval_ = eval_
        self.nrows = nrows    # unique rows across all slots


def _split_item(it, cap, roomw):
    """Take the largest slot-prefix of `it` fitting (cap slots, roomw rows).
    Returns (piece, remainder|None)."""
    rows_seen = set()
    k = 0
    while k < len(it.cols) and k < cap:
        new = rows_seen | set(it.erow[it.eoff[k] : it.eoff[k + 1]].tolist())
        if len(new) > roomw:
            break
        rows_seen = new
        k += 1
    if k == 0:
        return None, it
    e1 = int(it.eoff[k])
    piece = _Item(
        it.cols[:k], it.eoff[: k + 1], it.erow[:e1], it.eval_[:e1], len(rows_seen)
    )
    if k == len(it.cols):
        return piece, None
    rerow = it.erow[e1:]
    rem = _Item(
        it.cols[k:], it.eoff[k:] - e1, rerow, it.eval_[e1:], len(np.unique(rerow))
    )
    return piece, rem


def _cluster_and_pack(rj, cj, vj):
    """Per (core, class): cluster rows sharing cols (capped union-find), then
    pack clusters into tiles of <=P slots / <=W rows with slot-granular
    splitting so tiles fill to exactly P.  A slot serves every clustered edge
    with that col (bval column gets several nonzeros).

    Returns list of tiles: (slots_cols, edge_slot, edge_row_global, edge_val)."""
    pres, inv_r, deg = np.unique(rj, return_inverse=True, return_counts=True)
    nr = len(pres)
    if nr == 0:
        return []
    parent = np.arange(nr)
    szr = np.ones(nr, np.int64)
    sze = deg.copy().astype(np.int64)

    def find(a):
        while parent[a] != a:
            parent[a] = parent[parent[a]]
            a = parent[a]
        return a

    order = np.argsort(cj, kind="stable")
    sc = cj[order]
    sr = inv_r[order]
    same = np.nonzero(sc[1:] == sc[:-1])[0]
    for i in same:
        a, b = find(sr[i]), find(sr[i + 1])
        if a == b:
            continue
        if szr[a] + szr[b] <= MAXR and sze[a] + sze[b] <= MAXE:
            parent[b] = a
            szr[a] += szr[b]
            sze[a] += sze[b]
    root = np.fromiter((find(i) for i in range(nr)), np.int64, nr)

    corder = np.argsort(root, kind="stable")
    croots = root[corder]
    starts = np.nonzero(np.concatenate([[True], croots[1:] != croots[:-1]]))[0]
    bounds = np.concatenate([starts, [nr]])

    er_order = np.argsort(inv_r, kind="stable")
    row_off = np.concatenate([[0], np.cumsum(deg)])
    items = []
    for gi in range(len(starts)):
        ranks = corder[bounds[gi] : bounds[gi + 1]]
        eidx = np.concatenate([er_order[row_off[r] : row_off[r + 1]] for r in ranks])
        ccols = cj[eidx]
        so = np.argsort(ccols, kind="stable")  # group edges by col
        scols = ccols[so]
        uniq_mask = np.concatenate([[True], scols[1:] != scols[:-1]])
        uniq = scols[uniq_mask]
        eoff = np.concatenate([np.nonzero(uniq_mask)[0], [len(scols)]])
        items.append(
            _Item(uniq, eoff, pres[inv_r[eidx[so]]], vj[eidx[so]], len(ranks))
        )

    # pack: by-size buckets, prefer largest whole item, split to fill
    maxd = max(len(it.cols) for it in items)
    by_size = [[] for _ in range(maxd + 1)]
    for it in items:
        by_size[len(it.cols)].append(it)
    navail = len(items)
    tiles = []
    while navail:
        cap = P
        roomw = W
        pieces = []
        cur = maxd
        while navail and cap > 0 and roomw > 0:
            while cur > 0 and not by_size[cur]:
                cur -= 1
            if cur == 0:
                break
            dd = min(cap, cur)
            while dd > 0 and not by_size[dd]:
                dd -= 1
            placed = False
            if dd > 0:
                cand = by_size[dd][-1]
                if cand.nrows <= roomw:
                    by_size[dd].pop()
                    navail -= 1
                    pieces.append(cand)
                    cap -= len(cand.cols)
                    roomw -= cand.nrows
                    placed = True
            if not placed:
                # split the largest remaining item to fill the tile
                it = by_size[cur].pop()
                piece, rem = _split_item(it, cap, roomw)
                if rem is not None:
                    by_size[len(rem.cols)].append(rem)
                    if len(rem.cols) > cur:
                        cur = len(rem.cols)
                else:
                    navail -= 1
                if piece is None:
                    break  # tile can't host even one slot of it
                pieces.append(piece)
                cap -= len(piece.cols)
                roomw -= piece.nrows
        if not pieces:
            break
        slots_cols = np.concatenate([p.cols for p in pieces])
        e_slot = np.concatenate(
            [
                np.repeat(np.arange(len(p.cols)) + so, np.diff(p.eoff))
                for p, so in zip(
                    pieces, np.cumsum([0] + [len(p.cols) for p in pieces])[:-1]
                )
            ]
        )
        e_row = np.concatenate([p.erow for p in pieces])
        e_val = np.concatenate([p.eval_ for p in pieces])
        tiles.append((slots_cols, e_slot, e_row, e_val))
    return tiles


def _prepare_core(rows, cols, vals, core):
    lo = core * RPC
    e0, e1 = np.searchsorted(rows, [lo, lo + RPC])
    r = (np.asarray(rows[e0:e1]) - lo).astype(np.int64)
    c = np.asarray(cols[e0:e1]).astype(np.int64)
    v = np.asarray(vals[e0:e1]).astype(np.float32)
    out = []
    for j in range(NCLS):
        m = c % NCLS == j
        out.append(_cluster_and_pack(r[m], c[m].astype(np.int32), v[m]))
    return out


def _nsb_for(per_core):
    need = np.zeros(NCLS, np.int64)
    for pc in per_core:
        for j in range(NCLS):
            need[j] = max(need[j], len(pc[j]))
    nsb = 1
    while True:
        nt = nsb * TPS
        if all((nt + NCLS - 1 - j) // NCLS >= need[j] for j in range(NCLS)):
            return nsb
        nsb += 1


def _assemble_core(per_core_k, nsb):
    nt = nsb * TPS
    idx_all = np.zeros((128, nt), np.int32)
    bval = np.zeros((128, nsb * 512), np.float32)
    prow_l, ppos_l = [], []
    for j in range(NCLS):
        tiles = per_core_k[j]
        tids = np.arange(j, nt, NCLS)
        assert len(tiles) <= len(tids), (len(tiles), len(tids), j)
        for (scols, e_slot, e_row, e_val), t in zip(tiles, tids):
            s, tl = divmod(t, TPS)
            ns = len(scols)
            idx_all[:ns, t] = scols
            rws, e_rloc = np.unique(e_row, return_inverse=True)
            assert len(rws) <= W and ns <= P
            np.add.at(bval, (e_slot, s * 512 + tl * W + e_rloc), e_val)
            prow_l.append(rws)
            cc = tl * W + np.arange(len(rws))
            ppos_l.append((s * 128 + cc % 128) * 4 + cc // 128)
    prow = np.concatenate(prow_l) if prow_l else np.zeros(0, np.int64)
    ppos = np.concatenate(ppos_l) if ppos_l else np.zeros(0, np.int64)
    return idx_all, bval, prow, ppos


def _build_program(nsb):
    nt = nsb * TPS
    nc = bacc.Bacc("TRN2", target_bir_lowering=False, debug=False)
    x = nc.dram_tensor("x", [N_NODES, F], f32, kind="ExternalInput")
    idx = nc.dram_tensor("idx", [128, nt], i32, kind="ExternalInput")
    bval = nc.dram_tensor("bval", [128, nsb * 512], f32, kind="ExternalInput")
    w = nc.dram_tensor("w", [F, F], f32, kind="ExternalInput")
    out = nc.dram_tensor("out", [nsb, 128, 128], f32, kind="ExternalOutput")

    with tile.TileContext(nc) as tc:
        with (
            tc.tile_pool(name="const", bufs=1) as cpool,
            tc.tile_pool(name="bv", bufs=4) as bvpool,
            tc.tile_pool(name="g", bufs=6) as gpool,
            tc.tile_pool(name="zt", bufs=4, space="PSUM") as ztpool,
            tc.tile_pool(name="po", bufs=2, space="PSUM") as popool,
            tc.tile_pool(name="ev", bufs=4) as evpool,
        ):
            wt = cpool.tile([F, F], f32)
            nc.sync.dma_start(wt[:], w[:])
            # all gather offsets stay SBUF-resident: zero per-tile load stalls
            idxr = cpool.tile([128, nt], i32)
            nc.sync.dma_start(idxr[:], idx[:])
            for s in range(nsb):
                gt = gpool.tile([128, TPS * F], f32, tag="g")
                for tl in range(TPS):
                    t = s * TPS + tl
                    nc.gpsimd.indirect_dma_start(
                        out=gt[:, tl * F : (tl + 1) * F],
                        out_offset=None,
                        in_=x[:],
                        in_offset=IndirectOffsetOnAxis(
                            ap=idxr[:, t : t + 1], axis=0
                        ),
                    )
                bvt = bvpool.tile([128, 512], f32, tag="bv")
                nc.sync.dma_start(bvt[:], bval[:, s * 512 : (s + 1) * 512])
                zt = ztpool.tile([F, 512], f32, tag="zt")
                for tl in range(TPS):
                    nc.tensor.matmul(
                        out=zt[:, tl * W : (tl + 1) * W],
                        lhsT=gt[:, tl * F : (tl + 1) * F],
                        rhs=bvt[:, tl * W : (tl + 1) * W],
                        start=True,
                        stop=True,
                    )
                zsb = evpool.tile([F, 512], f32, tag="zsb")
                nc.vector.tensor_copy(zsb[:], zt[:])
                pot = popool.tile([128, 4 * F], f32, tag="po")
                for ci in range(4):
                    nc.tensor.matmul(
                        out=pot[:, ci * F : (ci + 1) * F],
                        lhsT=zsb[:, ci * 128 : (ci + 1) * 128],
                        rhs=wt[:],
                        start=True,
                        stop=True,
                    )
                ott = evpool.tile([128, 4 * F], f32, tag="ot")
                nc.vector.tensor_copy(ott[:], pot[:])
                nc.scalar.dma_start(out[s], ott[:])
    nc.compile()
    return nc


def prepare(x, rows, cols, vals, weight):
    """Host packing + compile.  Returns (nc, in_maps, maps, nsb)."""
    x = np.ascontiguousarray(np.asarray(x, dtype=np.float32))
    rows = np.asarray(rows)
    cols = np.asarray(cols)
    vals = np.asarray(vals, dtype=np.float32)
    weight = np.ascontiguousarray(np.asarray(weight, dtype=np.float32))

    per_core = [_prepare_core(rows, cols, vals, k) for k in range(N_CORES)]
    nsb = _nsb_for(per_core)

    if nsb not in _compiled_cache:
        _compiled_cache[nsb] = _build_program(nsb)
    nc = _compiled_cache[nsb]

    in_maps = []
    maps = []
    for k in range(N_CORES):
        idx_all, bval, prow, ppos = _assemble_core(per_core[k], nsb)
        maps.append((prow, ppos))
        in_maps.append({"x": x, "idx": idx_all, "bval": bval, "w": weight})
    return nc, in_maps, maps, nsb


def gather_output(res, maps, nsb):
    out_full = np.zeros((N_NODES, F), np.float32)
    for k in range(N_CORES):
        dev = np.asarray(res.results[k]["out"], dtype=np.float32)
        dev = dev.reshape(nsb * 128 * 4, 32)
        prow, ppos = maps[k]
        np.add.at(out_full, k * RPC + prow, dev[ppos])
    return out_full


def kernel(x, rows, cols, vals, weight):
    nc, in_maps, maps, nsb = prepare(x, rows, cols, vals, weight)
    res = run_bass_kernel_spmd(nc, in_maps, list(range(N_CORES)))
    return gather_output(res, maps, nsb)


# revision 15
# speedup vs baseline: 1.3160x; 1.3160x over previous
"""GCNConv (COO SpMM + feature transform) distributed over 8 NeuronCores.

out = segment_sum(x[cols] * vals, rows) @ weight

Strategy (1D row partition per the CAGNET-style hint): core k owns dest rows
[12500k, 12500(k+1)) and the edges targeting them; x and the 32x32 weight are
replicated.

The kernel is bound by GpSimd SWDGE descriptor generation (~8.6ns per
gathered row, serial on the Pool engine — measured; the cost-model's
994ns-fixed + 0.34ns/desc is wrong for the indirect/gather ucode paths, so
batching descriptors into fewer instructions does NOT help).  The design
therefore minimizes DESCRIPTOR COUNT and keeps the Pool engine saturated:
 - Per (core, class=col%8): rows sharing class-cols are clustered (capped
   union-find) so edges with an identical col share ONE gather slot (the
   bval one-hot column for that slot simply has several nonzeros).  The
   class split is scale-invariant in col-collision rate (lambda=2 per
   class-col at any NCLS) but fewer edges/row/class means deeper clusters:
   NCLS=8 realizes ~36% slot dedup vs ~21% at NCLS=4 (NCLS=16 dedups
   further but the f32 segsum matmuls then become PE-bound).
 - Tiles: 128 slots (one indirect DMA each), <=W packed dest rows; TPS
   tiles accumulate into one PSUM bank zt[32, 512] (every column written,
   pad tiles write zeros).  W=128/TPS=4 keeps the row side (rows appear
   once per class they have edges in) from becoming the binding constraint
   while keeping the PE segsum matmuls under the Pool-engine shadow.
 - All gather offsets live SBUF-resident (one load at startup), so the
   gathers issue back-to-back with no per-tile dependency stalls.
 - Eviction applies the weight (4 f32 matmuls) and stores a contiguous
   [128, 128] f32 block per super-block; the host un-permutes packed rows
   (np.add.at sums rows split across tiles/classes).
 - Program shape depends only on NSB -> one NEFF runs SPMD on all 8 cores.
"""

import os
import sys
import tempfile
import types

import numpy as np
import ml_dtypes

# A transiently-wedged device can leave a poisoned NEFF in the shared neuron
# compile cache, making every later invocation with the same cache key crash
# (observed: NRT_EXEC_UNIT_UNRECOVERABLE on known-good programs).  Compiling
# is only a few seconds here, so use a fresh per-process cache instead.
os.environ["NEURON_COMPILE_CACHE_URL"] = tempfile.mkdtemp(prefix="neuron-cc-cache-")


def _install_ntff_hook_shim():
    """bass_utils' axon trace path imports antenv.axon_hooks, which this
    container image lacks.  Provide it (with the real ctypes-based profiler
    hook when available) so BASS_TRACE=1 in the environment doesn't crash."""
    if "antenv.axon_hooks" in sys.modules:
        return
    mod = types.ModuleType("antenv.axon_hooks")
    _h = [None]
    mod.set_axon_ntff_profile_hook = lambda h: _h.__setitem__(0, h)
    mod.get_axon_ntff_profile_hook = lambda: _h[0]
    sys.modules["antenv.axon_hooks"] = mod
    try:
        from trn_agent_boot.trn_boot import _ntff_profile_via_ctypes

        mod.set_axon_ntff_profile_hook(
            _ntff_profile_via_ctypes("/opt/axon/libaxon_pjrt.so")
        )
    except Exception:
        pass


_install_ntff_hook_shim()

import concourse.bass as bass
import concourse.mybir as mybir
import concourse.tile as tile
from concourse import bacc
from concourse.bass import IndirectOffsetOnAxis
from concourse.bass_utils import run_bass_kernel_spmd

N_NODES = 100_000
N_CORES = 8
RPC = N_NODES // N_CORES  # rows per core
F = 32
NCLS = 16                 # classes = col % 16
W = 256                   # packed output cols per tile (max rows per tile)
TPS = 2                   # tiles per super-block (2*256 = 512 PSUM cols)
P = 128                   # slots per tile

MAXR = 250                # cluster cap: rows
MAXE = 512                # cluster cap: edges

f32 = mybir.dt.float32
bf16 = mybir.dt.bfloat16
i32 = mybir.dt.int32

_compiled_cache = {}


class _Item:
    """A packable cluster piece: `cols` (one gather slot per unique col) and
    CSR slot->edges arrays (row = GLOBAL dest row)."""

    __slots__ = ("cols", "eoff", "erow", "eval_", "nrows")

    def __init__(self, cols, eoff, erow, eval_, nrows):
        self.cols = cols
        self.eoff = eoff      # [len(cols)+1] edge offsets per slot
        self.erow = erow      # edge rows, grouped by slot
        self.eval_ = eval_
        self.nrows = nrows    # unique rows across all slots


def _split_item(it, cap, roomw):
    """Take the largest slot-prefix of `it` fitting (cap slots, roomw rows).
    Returns (piece, remainder|None)."""
    rows_seen = set()
    k = 0
    while k < len(it.cols) and k < cap:
        new = rows_seen | set(it.erow[it.eoff[k] : it.eoff[k + 1]].tolist())
        if len(new) > roomw:
            break
        rows_seen = new
        k += 1
    if k == 0:
        return None, it
    e1 = int(it.eoff[k])
    piece = _Item(
        it.cols[:k], it.eoff[: k + 1], it.erow[:e1], it.eval_[:e1], len(rows_seen)
    )
    if k == len(it.cols):
        return piece, None
    rerow = it.erow[e1:]
    rem = _Item(
        it.cols[k:], it.eoff[k:] - e1, rerow, it.eval_[e1:], len(np.unique(rerow))
    )
    return piece, rem


def _cluster_and_pack(rj, cj, vj):
    """Per (core, class): cluster rows sharing cols (capped union-find), then
    pack clusters into tiles of <=P slots / <=W rows with slot-granular
    splitting so tiles fill to exactly P.  A slot serves every clustered edge
    with that col (bval column gets several nonzeros).

    Returns list of tiles: (slots_cols, edge_slot, edge_row_global, edge_val)."""
    pres, inv_r, deg = np.unique(rj, return_inverse=True, return_counts=True)
    nr = len(pres)
    if nr == 0:
        return []
    parent = np.arange(nr)
    szr = np.ones(nr, np.int64)
    sze = deg.copy().astype(np.int64)

    def find(a):
        while parent[a] != a:
            parent[a] = parent[parent[a]]
            a = parent[a]
        return a

    order = np.argsort(cj, kind="stable")
    sc = cj[order]
    sr = inv_r[order]
    same = np.nonzero(sc[1:] == sc[:-1])[0]
    for i in same:
        a, b = find(sr[i]), find(sr[i + 1])
        if a == b:
            continue
        if szr[a] + szr[b] <= MAXR and sze[a] + sze[b] <= MAXE:
            parent[b] = a
            szr[a] += szr[b]
            sze[a] += sze[b]
    root = np.fromiter((find(i) for i in range(nr)), np.int64, nr)

    corder = np.argsort(root, kind="stable")
    croots = root[corder]
    starts = np.nonzero(np.concatenate([[True], croots[1:] != croots[:-1]]))[0]
    bounds = np.concatenate([starts, [nr]])

    er_order = np.argsort(inv_r, kind="stable")
    row_off = np.concatenate([[0], np.cumsum(deg)])
    items = []
    for gi in range(len(starts)):
        ranks = corder[bounds[gi] : bounds[gi + 1]]
        eidx = np.concatenate([er_order[row_off[r] : row_off[r + 1]] for r in ranks])
        ccols = cj[eidx]
        so = np.argsort(ccols, kind="stable")  # group edges by col
        scols = ccols[so]
        uniq_mask = np.concatenate([[True], scols[1:] != scols[:-1]])
        uniq = scols[uniq_mask]
        eoff = np.concatenate([np.nonzero(uniq_mask)[0], [len(scols)]])
        items.append(
            _Item(uniq, eoff, pres[inv_r[eidx[so]]], vj[eidx[so]], len(ranks))
        )

    # pack: by-size buckets, prefer largest whole item, split to fill
    maxd = max(len(it.cols) for it in items)
    by_size = [[] for _ in range(maxd + 1)]
    for it in items:
        by_size[len(it.cols)].append(it)
    navail = len(items)
    tiles = []
    while navail:
        cap = P
        roomw = W
        pieces = []
        cur = maxd
        while navail and cap > 0 and roomw > 0:
            while cur > 0 and not by_size[cur]:
                cur -= 1
            if cur == 0:
                break
            dd = min(cap, cur)
            while dd > 0 and not by_size[dd]:
                dd -= 1
            placed = False
            if dd > 0:
                cand = by_size[dd][-1]
                if cand.nrows <= roomw:
                    by_size[dd].pop()
                    navail -= 1
                    pieces.append(cand)
                    cap -= len(cand.cols)
                    roomw -= cand.nrows
                    placed = True
            if not placed:
                # split the largest remaining item to fill the tile
                it = by_size[cur].pop()
                piece, rem = _split_item(it, cap, roomw)
                if rem is not None:
                    by_size[len(rem.cols)].append(rem)
                    if len(rem.cols) > cur:
                        cur = len(rem.cols)
                else:
                    navail -= 1
                if piece is None:
                    break  # tile can't host even one slot of it
                pieces.append(piece)
                cap -= len(piece.cols)
                roomw -= piece.nrows
        if not pieces:
            break
        slots_cols = np.concatenate([p.cols for p in pieces])
        e_slot = np.concatenate(
            [
                np.repeat(np.arange(len(p.cols)) + so, np.diff(p.eoff))
                for p, so in zip(
                    pieces, np.cumsum([0] + [len(p.cols) for p in pieces])[:-1]
                )
            ]
        )
        e_row = np.concatenate([p.erow for p in pieces])
        e_val = np.concatenate([p.eval_ for p in pieces])
        tiles.append((slots_cols, e_slot, e_row, e_val))
    return tiles


def _prepare_core(rows, cols, vals, core):
    lo = core * RPC
    e0, e1 = np.searchsorted(rows, [lo, lo + RPC])
    r = (np.asarray(rows[e0:e1]) - lo).astype(np.int64)
    c = np.asarray(cols[e0:e1]).astype(np.int64)
    v = np.asarray(vals[e0:e1]).astype(np.float32)
    out = []
    for j in range(NCLS):
        m = c % NCLS == j
        out.append(_cluster_and_pack(r[m], c[m].astype(np.int32), v[m]))
    return out


def _nsb_for(per_core):
    need = np.zeros(NCLS, np.int64)
    for pc in per_core:
        for j in range(NCLS):
            need[j] = max(need[j], len(pc[j]))
    nsb = 1
    while True:
        nt = nsb * TPS
        if all((nt + NCLS - 1 - j) // NCLS >= need[j] for j in range(NCLS)):
            return nsb
        nsb += 1


def _assemble_core(per_core_k, nsb):
    nt = nsb * TPS
    idx_all = np.zeros((128, nt), np.int32)
    bval = np.zeros((128, nsb * 512), np.float32)
    prow_l, ppos_l = [], []
    for j in range(NCLS):
        tiles = per_core_k[j]
        tids = np.arange(j, nt, NCLS)
        assert len(tiles) <= len(tids), (len(tiles), len(tids), j)
        for (scols, e_slot, e_row, e_val), t in zip(tiles, tids):
            s, tl = divmod(t, TPS)
            ns = len(scols)
            idx_all[:ns, t] = scols
            rws, e_rloc = np.unique(e_row, return_inverse=True)
            assert len(rws) <= W and ns <= P
            np.add.at(bval, (e_slot, s * 512 + tl * W + e_rloc), e_val)
            prow_l.append(rws)
            cc = tl * W + np.arange(len(rws))
            ppos_l.append((s * 128 + cc % 128) * 4 + cc // 128)
    prow = np.concatenate(prow_l) if prow_l else np.zeros(0, np.int64)
    ppos = np.concatenate(ppos_l) if ppos_l else np.zeros(0, np.int64)
    return idx_all, bval.astype(ml_dtypes.bfloat16), prow, ppos


def _build_program(nsb):
    nt = nsb * TPS
    nc = bacc.Bacc("TRN2", target_bir_lowering=False, debug=False)
    x = nc.dram_tensor("x", [N_NODES, F], f32, kind="ExternalInput")
    idx = nc.dram_tensor("idx", [128, nt], i32, kind="ExternalInput")
    bval = nc.dram_tensor("bval", [128, nsb * 512], bf16, kind="ExternalInput")
    w = nc.dram_tensor("w", [F, F], bf16, kind="ExternalInput")
    out = nc.dram_tensor("out", [nsb, 128, 128], f32, kind="ExternalOutput")

    with tile.TileContext(nc) as tc:
        with (
            tc.tile_pool(name="const", bufs=1) as cpool,
            tc.tile_pool(name="bv", bufs=4) as bvpool,
            tc.tile_pool(name="g", bufs=6) as gpool,
            tc.tile_pool(name="zt", bufs=4, space="PSUM") as ztpool,
            tc.tile_pool(name="po", bufs=2, space="PSUM") as popool,
            tc.tile_pool(name="ev", bufs=4) as evpool,
        ):
            wt = cpool.tile([F, F], bf16)
            nc.sync.dma_start(wt[:], w[:])
            # all gather offsets stay SBUF-resident: zero per-tile load stalls
            idxr = cpool.tile([128, nt], i32)
            nc.sync.dma_start(idxr[:], idx[:])
            for s in range(nsb):
                gt = gpool.tile([128, TPS * F], f32, tag="g")
                for tl in range(TPS):
                    t = s * TPS + tl
                    nc.gpsimd.indirect_dma_start(
                        out=gt[:, tl * F : (tl + 1) * F],
                        out_offset=None,
                        in_=x[:],
                        in_offset=IndirectOffsetOnAxis(
                            ap=idxr[:, t : t + 1], axis=0
                        ),
                    )
                bvt = bvpool.tile([128, 512], bf16, tag="bv")
                nc.sync.dma_start(bvt[:], bval[:, s * 512 : (s + 1) * 512])
                gtb = gpool.tile([128, TPS * F], bf16, tag="gtb")
                nc.vector.tensor_copy(gtb[:], gt[:])
                zt = ztpool.tile([F, 512], f32, tag="zt")
                for tl in range(TPS):
                    nc.tensor.matmul(
                        out=zt[:, tl * W : (tl + 1) * W],
                        lhsT=gtb[:, tl * F : (tl + 1) * F],
                        rhs=bvt[:, tl * W : (tl + 1) * W],
                        start=True,
                        stop=True,
                    )
                zsb = evpool.tile([F, 512], bf16, tag="zsb")
                nc.scalar.copy(zsb[:], zt[:])
                pot = popool.tile([128, 4 * F], f32, tag="po")
                for ci in range(4):
                    nc.tensor.matmul(
                        out=pot[:, ci * F : (ci + 1) * F],
                        lhsT=zsb[:, ci * 128 : (ci + 1) * 128],
                        rhs=wt[:],
                        start=True,
                        stop=True,
                    )
                ott = evpool.tile([128, 4 * F], f32, tag="ot")
                nc.vector.tensor_copy(ott[:], pot[:])
                nc.scalar.dma_start(out[s], ott[:])
    nc.compile()
    return nc


def prepare(x, rows, cols, vals, weight):
    """Host packing + compile.  Returns (nc, in_maps, maps, nsb)."""
    x = np.ascontiguousarray(np.asarray(x, dtype=np.float32))
    rows = np.asarray(rows)
    cols = np.asarray(cols)
    vals = np.asarray(vals, dtype=np.float32)
    weight = np.ascontiguousarray(np.asarray(weight, dtype=np.float32))

    per_core = [_prepare_core(rows, cols, vals, k) for k in range(N_CORES)]
    nsb = _nsb_for(per_core)

    if nsb not in _compiled_cache:
        _compiled_cache[nsb] = _build_program(nsb)
    nc = _compiled_cache[nsb]

    in_maps = []
    maps = []
    for k in range(N_CORES):
        idx_all, bval, prow, ppos = _assemble_core(per_core[k], nsb)
        maps.append((prow, ppos))
        in_maps.append(
            {"x": x, "idx": idx_all, "bval": bval, "w": weight.astype(ml_dtypes.bfloat16)}
        )
    return nc, in_maps, maps, nsb


def gather_output(res, maps, nsb):
    out_full = np.zeros((N_NODES, F), np.float32)
    for k in range(N_CORES):
        dev = np.asarray(res.results[k]["out"], dtype=np.float32)
        dev = dev.reshape(nsb * 128 * 4, 32)
        prow, ppos = maps[k]
        np.add.at(out_full, k * RPC + prow, dev[ppos])
    return out_full


def kernel(x, rows, cols, vals, weight):
    nc, in_maps, maps, nsb = prepare(x, rows, cols, vals, weight)
    res = run_bass_kernel_spmd(nc, in_maps, list(range(N_CORES)))
    return gather_output(res, maps, nsb)


# revision 18
# speedup vs baseline: 1.5386x; 1.1691x over previous
"""GCNConv (COO SpMM + feature transform) distributed over 8 NeuronCores.

out = segment_sum(x[cols] * vals, rows) @ weight

Strategy (1D row partition per the CAGNET-style hint): core k owns dest rows
[12500k, 12500(k+1)) and the edges targeting them; x and the 32x32 weight are
replicated.

The kernel is bound by GpSimd SWDGE descriptor generation (~8.6ns per
gathered row, serial on the Pool engine — measured; the cost-model's
994ns-fixed + 0.34ns/desc is wrong for the indirect/gather ucode paths, so
batching descriptors into fewer instructions does NOT help).  The design
therefore minimizes DESCRIPTOR COUNT and keeps the Pool engine saturated:
 - Per (core, class=col%16): rows sharing class-cols are clustered (capped
   union-find) so edges with an identical col share ONE gather slot (the
   bval one-hot column for that slot simply has several nonzeros).  The
   class split is scale-invariant in col-collision rate (lambda=2 per
   class-col at any NCLS) but fewer edges/row/class means deeper clusters:
   NCLS=16 realizes ~50% slot dedup vs ~21% at NCLS=4 (~104k descriptors
   per core vs ~200k one-per-edge).
 - Tiles: 128 slots (one indirect DMA each), <=W packed dest rows; TPS
   tiles accumulate into one PSUM bank zt[32, 512] (every column written,
   pad tiles write zeros).  W=256/TPS=2 keeps the row side (rows appear
   once per class they have edges in) from becoming the binding constraint;
   the gather DMA casts x rows to bf16 in flight so the wide segsum matmuls stay
   far below the Pool-engine shadow (f32 at W=256 was PE-bound).
 - All gather offsets live SBUF-resident (one load at startup), so the
   gathers issue back-to-back with no per-tile dependency stalls.
 - Eviction applies the weight (4 bf16 matmuls) and stores a contiguous
   [128, 128] f32 block per super-block; the host un-permutes packed rows
   (np.add.at sums rows split across tiles/classes).
 - Program shape depends only on NSB -> one NEFF runs SPMD on all 8 cores.
"""

import os
import sys
import tempfile
import types

import numpy as np
import ml_dtypes

# A transiently-wedged device can leave a poisoned NEFF in the shared neuron
# compile cache, making every later invocation with the same cache key crash
# (observed: NRT_EXEC_UNIT_UNRECOVERABLE on known-good programs).  Compiling
# is only a few seconds here, so use a fresh per-process cache instead.
os.environ["NEURON_COMPILE_CACHE_URL"] = tempfile.mkdtemp(prefix="neuron-cc-cache-")


def _install_ntff_hook_shim():
    """bass_utils' axon trace path imports antenv.axon_hooks, which this
    container image lacks.  Provide it (with the real ctypes-based profiler
    hook when available) so BASS_TRACE=1 in the environment doesn't crash."""
    if "antenv.axon_hooks" in sys.modules:
        return
    mod = types.ModuleType("antenv.axon_hooks")
    _h = [None]
    mod.set_axon_ntff_profile_hook = lambda h: _h.__setitem__(0, h)
    mod.get_axon_ntff_profile_hook = lambda: _h[0]
    sys.modules["antenv.axon_hooks"] = mod
    try:
        from trn_agent_boot.trn_boot import _ntff_profile_via_ctypes

        mod.set_axon_ntff_profile_hook(
            _ntff_profile_via_ctypes("/opt/axon/libaxon_pjrt.so")
        )
    except Exception:
        pass


_install_ntff_hook_shim()

import concourse.bass as bass
import concourse.mybir as mybir
import concourse.tile as tile
from concourse import bacc
from concourse.bass import IndirectOffsetOnAxis
from concourse.bass_utils import run_bass_kernel_spmd

N_NODES = 100_000
N_CORES = 8
RPC = N_NODES // N_CORES  # rows per core
F = 32
NCLS = 16                 # classes = col % 16
W = 256                   # packed output cols per tile (max rows per tile)
TPS = 2                   # tiles per super-block (2*256 = 512 PSUM cols)
P = 128                   # slots per tile

MAXR = 250                # cluster cap: rows
MAXE = 512                # cluster cap: edges

f32 = mybir.dt.float32
bf16 = mybir.dt.bfloat16
i32 = mybir.dt.int32

_compiled_cache = {}


class _Item:
    """A packable cluster piece: `cols` (one gather slot per unique col) and
    CSR slot->edges arrays (row = GLOBAL dest row)."""

    __slots__ = ("cols", "eoff", "erow", "eval_", "nrows")

    def __init__(self, cols, eoff, erow, eval_, nrows):
        self.cols = cols
        self.eoff = eoff      # [len(cols)+1] edge offsets per slot
        self.erow = erow      # edge rows, grouped by slot
        self.eval_ = eval_
        self.nrows = nrows    # unique rows across all slots


def _split_item(it, cap, roomw):
    """Take the largest slot-prefix of `it` fitting (cap slots, roomw rows).
    Returns (piece, remainder|None)."""
    rows_seen = set()
    k = 0
    while k < len(it.cols) and k < cap:
        new = rows_seen | set(it.erow[it.eoff[k] : it.eoff[k + 1]].tolist())
        if len(new) > roomw:
            break
        rows_seen = new
        k += 1
    if k == 0:
        return None, it
    e1 = int(it.eoff[k])
    piece = _Item(
        it.cols[:k], it.eoff[: k + 1], it.erow[:e1], it.eval_[:e1], len(rows_seen)
    )
    if k == len(it.cols):
        return piece, None
    rerow = it.erow[e1:]
    rem = _Item(
        it.cols[k:], it.eoff[k:] - e1, rerow, it.eval_[e1:], len(np.unique(rerow))
    )
    return piece, rem


def _cluster_and_pack(rj, cj, vj):
    """Per (core, class): cluster rows sharing cols (capped union-find), then
    pack clusters into tiles of <=P slots / <=W rows with slot-granular
    splitting so tiles fill to exactly P.  A slot serves every clustered edge
    with that col (bval column gets several nonzeros).

    Returns list of tiles: (slots_cols, edge_slot, edge_row_global, edge_val)."""
    pres, inv_r, deg = np.unique(rj, return_inverse=True, return_counts=True)
    nr = len(pres)
    if nr == 0:
        return []
    parent = np.arange(nr)
    szr = np.ones(nr, np.int64)
    sze = deg.copy().astype(np.int64)

    def find(a):
        while parent[a] != a:
            parent[a] = parent[parent[a]]
            a = parent[a]
        return a

    order = np.argsort(cj, kind="stable")
    sc = cj[order]
    sr = inv_r[order]
    same = np.nonzero(sc[1:] == sc[:-1])[0]
    for i in same:
        a, b = find(sr[i]), find(sr[i + 1])
        if a == b:
            continue
        if szr[a] + szr[b] <= MAXR and sze[a] + sze[b] <= MAXE:
            parent[b] = a
            szr[a] += szr[b]
            sze[a] += sze[b]
    root = np.fromiter((find(i) for i in range(nr)), np.int64, nr)

    corder = np.argsort(root, kind="stable")
    croots = root[corder]
    starts = np.nonzero(np.concatenate([[True], croots[1:] != croots[:-1]]))[0]
    bounds = np.concatenate([starts, [nr]])

    er_order = np.argsort(inv_r, kind="stable")
    row_off = np.concatenate([[0], np.cumsum(deg)])
    items = []
    for gi in range(len(starts)):
        ranks = corder[bounds[gi] : bounds[gi + 1]]
        eidx = np.concatenate([er_order[row_off[r] : row_off[r + 1]] for r in ranks])
        ccols = cj[eidx]
        so = np.argsort(ccols, kind="stable")  # group edges by col
        scols = ccols[so]
        uniq_mask = np.concatenate([[True], scols[1:] != scols[:-1]])
        uniq = scols[uniq_mask]
        eoff = np.concatenate([np.nonzero(uniq_mask)[0], [len(scols)]])
        items.append(
            _Item(uniq, eoff, pres[inv_r[eidx[so]]], vj[eidx[so]], len(ranks))
        )

    # pack: by-size buckets, prefer largest whole item, split to fill
    maxd = max(len(it.cols) for it in items)
    by_size = [[] for _ in range(maxd + 1)]
    for it in items:
        by_size[len(it.cols)].append(it)
    navail = len(items)
    tiles = []
    while navail:
        cap = P
        roomw = W
        pieces = []
        cur = maxd
        while navail and cap > 0 and roomw > 0:
            while cur > 0 and not by_size[cur]:
                cur -= 1
            if cur == 0:
                break
            dd = min(cap, cur)
            while dd > 0 and not by_size[dd]:
                dd -= 1
            placed = False
            if dd > 0:
                cand = by_size[dd][-1]
                if cand.nrows <= roomw:
                    by_size[dd].pop()
                    navail -= 1
                    pieces.append(cand)
                    cap -= len(cand.cols)
                    roomw -= cand.nrows
                    placed = True
            if not placed:
                # split the largest remaining item to fill the tile
                it = by_size[cur].pop()
                piece, rem = _split_item(it, cap, roomw)
                if rem is not None:
                    by_size[len(rem.cols)].append(rem)
                    if len(rem.cols) > cur:
                        cur = len(rem.cols)
                else:
                    navail -= 1
                if piece is None:
                    break  # tile can't host even one slot of it
                pieces.append(piece)
                cap -= len(piece.cols)
                roomw -= piece.nrows
        if not pieces:
            break
        slots_cols = np.concatenate([p.cols for p in pieces])
        e_slot = np.concatenate(
            [
                np.repeat(np.arange(len(p.cols)) + so, np.diff(p.eoff))
                for p, so in zip(
                    pieces, np.cumsum([0] + [len(p.cols) for p in pieces])[:-1]
                )
            ]
        )
        e_row = np.concatenate([p.erow for p in pieces])
        e_val = np.concatenate([p.eval_ for p in pieces])
        tiles.append((slots_cols, e_slot, e_row, e_val))
    return tiles


def _prepare_core(rows, cols, vals, core):
    lo = core * RPC
    e0, e1 = np.searchsorted(rows, [lo, lo + RPC])
    r = (np.asarray(rows[e0:e1]) - lo).astype(np.int64)
    c = np.asarray(cols[e0:e1]).astype(np.int64)
    v = np.asarray(vals[e0:e1]).astype(np.float32)
    out = []
    for j in range(NCLS):
        m = c % NCLS == j
        out.append(_cluster_and_pack(r[m], c[m].astype(np.int32), v[m]))
    return out


def _nsb_for(per_core):
    need = np.zeros(NCLS, np.int64)
    for pc in per_core:
        for j in range(NCLS):
            need[j] = max(need[j], len(pc[j]))
    nsb = 1
    while True:
        nt = nsb * TPS
        if all((nt + NCLS - 1 - j) // NCLS >= need[j] for j in range(NCLS)):
            return nsb
        nsb += 1


def _assemble_core(per_core_k, nsb):
    nt = nsb * TPS
    idx_all = np.zeros((128, nt), np.int32)
    bval = np.zeros((128, nsb * 512), np.float32)
    prow_l, ppos_l = [], []
    for j in range(NCLS):
        tiles = per_core_k[j]
        tids = np.arange(j, nt, NCLS)
        assert len(tiles) <= len(tids), (len(tiles), len(tids), j)
        for (scols, e_slot, e_row, e_val), t in zip(tiles, tids):
            s, tl = divmod(t, TPS)
            ns = len(scols)
            idx_all[:ns, t] = scols
            rws, e_rloc = np.unique(e_row, return_inverse=True)
            assert len(rws) <= W and ns <= P
            np.add.at(bval, (e_slot, s * 512 + tl * W + e_rloc), e_val)
            prow_l.append(rws)
            cc = tl * W + np.arange(len(rws))
            ppos_l.append((s * 128 + cc % 128) * 4 + cc // 128)
    prow = np.concatenate(prow_l) if prow_l else np.zeros(0, np.int64)
    ppos = np.concatenate(ppos_l) if ppos_l else np.zeros(0, np.int64)
    return idx_all, bval.astype(ml_dtypes.bfloat16), prow, ppos


def _build_program(nsb):
    nt = nsb * TPS
    nc = bacc.Bacc("TRN2", target_bir_lowering=False, debug=False)
    x = nc.dram_tensor("x", [N_NODES, F], f32, kind="ExternalInput")
    idx = nc.dram_tensor("idx", [128, nt], i32, kind="ExternalInput")
    bval = nc.dram_tensor("bval", [128, nsb * 512], bf16, kind="ExternalInput")
    w = nc.dram_tensor("w", [F, F], bf16, kind="ExternalInput")
    out = nc.dram_tensor("out", [nsb, 128, 128], f32, kind="ExternalOutput")

    with tile.TileContext(nc) as tc:
        with (
            tc.tile_pool(name="const", bufs=1) as cpool,
            tc.tile_pool(name="bv", bufs=5) as bvpool,
            tc.tile_pool(name="g", bufs=8) as gpool,
            tc.tile_pool(name="zt", bufs=5, space="PSUM") as ztpool,
            tc.tile_pool(name="po", bufs=2, space="PSUM") as popool,
            tc.tile_pool(name="ev", bufs=5) as evpool,
        ):
            wt = cpool.tile([F, F], bf16)
            nc.sync.dma_start(wt[:], w[:])
            # all gather offsets stay SBUF-resident: zero per-tile load stalls
            idxr = cpool.tile([128, nt], i32)
            nc.sync.dma_start(idxr[:], idx[:])
            for s in range(nsb):
                gt = gpool.tile([128, TPS * F], bf16, tag="g")
                for tl in range(TPS):
                    t = s * TPS + tl
                    nc.gpsimd.indirect_dma_start(
                        out=gt[:, tl * F : (tl + 1) * F],
                        out_offset=None,
                        in_=x[:],
                        in_offset=IndirectOffsetOnAxis(
                            ap=idxr[:, t : t + 1], axis=0
                        ),
                    )
                bvt = bvpool.tile([128, 512], bf16, tag="bv")
                nc.sync.dma_start(bvt[:], bval[:, s * 512 : (s + 1) * 512])
                zt = ztpool.tile([F, 512], f32, tag="zt")
                for tl in range(TPS):
                    nc.tensor.matmul(
                        out=zt[:, tl * W : (tl + 1) * W],
                        lhsT=gt[:, tl * F : (tl + 1) * F],
                        rhs=bvt[:, tl * W : (tl + 1) * W],
                        start=True,
                        stop=True,
                    )
                zsb = evpool.tile([F, 512], bf16, tag="zsb")
                nc.scalar.copy(zsb[:], zt[:])
                pot = popool.tile([128, 4 * F], f32, tag="po")
                for ci in range(4):
                    nc.tensor.matmul(
                        out=pot[:, ci * F : (ci + 1) * F],
                        lhsT=zsb[:, ci * 128 : (ci + 1) * 128],
                        rhs=wt[:],
                        start=True,
                        stop=True,
                    )
                ott = evpool.tile([128, 4 * F], f32, tag="ot")
                nc.vector.tensor_copy(ott[:], pot[:])
                nc.scalar.dma_start(out[s], ott[:])
    nc.compile()
    return nc


def prepare(x, rows, cols, vals, weight):
    """Host packing + compile.  Returns (nc, in_maps, maps, nsb)."""
    x = np.ascontiguousarray(np.asarray(x, dtype=np.float32))
    rows = np.asarray(rows)
    cols = np.asarray(cols)
    vals = np.asarray(vals, dtype=np.float32)
    weight = np.ascontiguousarray(np.asarray(weight, dtype=np.float32))

    per_core = [_prepare_core(rows, cols, vals, k) for k in range(N_CORES)]
    nsb = _nsb_for(per_core)

    if nsb not in _compiled_cache:
        _compiled_cache[nsb] = _build_program(nsb)
    nc = _compiled_cache[nsb]

    in_maps = []
    maps = []
    for k in range(N_CORES):
        idx_all, bval, prow, ppos = _assemble_core(per_core[k], nsb)
        maps.append((prow, ppos))
        in_maps.append(
            {"x": x, "idx": idx_all, "bval": bval, "w": weight.astype(ml_dtypes.bfloat16)}
        )
    return nc, in_maps, maps, nsb


def gather_output(res, maps, nsb):
    out_full = np.zeros((N_NODES, F), np.float32)
    for k in range(N_CORES):
        dev = np.asarray(res.results[k]["out"], dtype=np.float32)
        dev = dev.reshape(nsb * 128 * 4, 32)
        prow, ppos = maps[k]
        np.add.at(out_full, k * RPC + prow, dev[ppos])
    return out_full


def kernel(x, rows, cols, vals, weight):
    nc, in_maps, maps, nsb = prepare(x, rows, cols, vals, weight)
    res = run_bass_kernel_spmd(nc, in_maps, list(range(N_CORES)))
    return gather_output(res, maps, nsb)


# revision 19
# speedup vs baseline: 1.7949x; 1.1666x over previous
"""GCNConv (COO SpMM + feature transform) distributed over 8 NeuronCores.

out = segment_sum(x[cols] * vals, rows) @ weight

Strategy (1D row partition per the CAGNET-style hint): core k owns dest rows
[12500k, 12500(k+1)) and the edges targeting them; x and the 32x32 weight are
replicated.

The kernel is bound by GpSimd SWDGE descriptor generation (~8.6ns per
gathered row, serial on the Pool engine — measured; the cost-model's
994ns-fixed + 0.34ns/desc is wrong for the indirect/gather ucode paths, so
batching descriptors into fewer instructions does NOT help).  The design
therefore minimizes DESCRIPTOR COUNT and keeps the Pool engine saturated:
 - Per (core, class=col%16): rows sharing class-cols are clustered (capped
   union-find) so edges with an identical col share ONE gather slot (the
   bval one-hot column for that slot simply has several nonzeros).  The
   class split is scale-invariant in col-collision rate (lambda=2 per
   class-col at any NCLS) but fewer edges/row/class means deeper clusters:
   NCLS=16 realizes ~50% slot dedup vs ~21% at NCLS=4 (~104k descriptors
   per core vs ~200k one-per-edge).
 - Tiles: 128 slots (one indirect DMA each), <=W packed dest rows; TPS
   tiles accumulate into one PSUM bank zt[32, 512] (every column written,
   pad tiles write zeros).  W=256/TPS=2 keeps the row side (rows appear
   once per class they have edges in) from becoming the binding constraint;
   the gather DMA casts x rows to bf16 in flight so the wide segsum matmuls stay
   far below the Pool-engine shadow (f32 at W=256 was PE-bound).
 - All gather offsets live SBUF-resident (one load at startup), so the
   gathers issue back-to-back with no per-tile dependency stalls.
 - Eviction applies the weight (4 bf16 matmuls) and stores a contiguous
   [128, 128] f32 block per super-block; the host un-permutes packed rows
   (np.add.at sums rows split across tiles/classes).
 - Program shape depends only on NSB -> one NEFF runs SPMD on all 8 cores.
"""

import os
import sys
import tempfile
import types

import numpy as np
import ml_dtypes

# A transiently-wedged device can leave a poisoned NEFF in the shared neuron
# compile cache, making every later invocation with the same cache key crash
# (observed: NRT_EXEC_UNIT_UNRECOVERABLE on known-good programs).  Compiling
# is only a few seconds here, so use a fresh per-process cache instead.
os.environ["NEURON_COMPILE_CACHE_URL"] = tempfile.mkdtemp(prefix="neuron-cc-cache-")


def _install_ntff_hook_shim():
    """bass_utils' axon trace path imports antenv.axon_hooks, which this
    container image lacks.  Provide it (with the real ctypes-based profiler
    hook when available) so BASS_TRACE=1 in the environment doesn't crash."""
    if "antenv.axon_hooks" in sys.modules:
        return
    mod = types.ModuleType("antenv.axon_hooks")
    _h = [None]
    mod.set_axon_ntff_profile_hook = lambda h: _h.__setitem__(0, h)
    mod.get_axon_ntff_profile_hook = lambda: _h[0]
    sys.modules["antenv.axon_hooks"] = mod
    try:
        from trn_agent_boot.trn_boot import _ntff_profile_via_ctypes

        mod.set_axon_ntff_profile_hook(
            _ntff_profile_via_ctypes("/opt/axon/libaxon_pjrt.so")
        )
    except Exception:
        pass


_install_ntff_hook_shim()

import concourse.bass as bass
import concourse.mybir as mybir
import concourse.tile as tile
from concourse import bacc
from concourse.bass import IndirectOffsetOnAxis
from concourse.bass_utils import run_bass_kernel_spmd

N_NODES = 100_000
N_CORES = 8
RPC = N_NODES // N_CORES  # rows per core
F = 32
NCLS = 32                 # classes = col % 32
W = 256                   # packed output cols per tile (max rows per tile)
TPS = 2                   # tiles per super-block (2*256 = 512 PSUM cols)
P = 128                   # slots per tile

MAXR = 250                # cluster cap: rows
MAXE = 512                # cluster cap: edges

f32 = mybir.dt.float32
bf16 = mybir.dt.bfloat16
i32 = mybir.dt.int32

_compiled_cache = {}


class _Item:
    """A packable cluster piece: `cols` (one gather slot per unique col) and
    CSR slot->edges arrays (row = GLOBAL dest row)."""

    __slots__ = ("cols", "eoff", "erow", "eval_", "nrows")

    def __init__(self, cols, eoff, erow, eval_, nrows):
        self.cols = cols
        self.eoff = eoff      # [len(cols)+1] edge offsets per slot
        self.erow = erow      # edge rows, grouped by slot
        self.eval_ = eval_
        self.nrows = nrows    # unique rows across all slots


def _split_item(it, cap, roomw):
    """Take the largest slot-prefix of `it` fitting (cap slots, roomw rows).
    Returns (piece, remainder|None)."""
    rows_seen = set()
    k = 0
    while k < len(it.cols) and k < cap:
        new = rows_seen | set(it.erow[it.eoff[k] : it.eoff[k + 1]].tolist())
        if len(new) > roomw:
            break
        rows_seen = new
        k += 1
    if k == 0:
        return None, it
    e1 = int(it.eoff[k])
    piece = _Item(
        it.cols[:k], it.eoff[: k + 1], it.erow[:e1], it.eval_[:e1], len(rows_seen)
    )
    if k == len(it.cols):
        return piece, None
    rerow = it.erow[e1:]
    rem = _Item(
        it.cols[k:], it.eoff[k:] - e1, rerow, it.eval_[e1:], len(np.unique(rerow))
    )
    return piece, rem


def _cluster_and_pack(rj, cj, vj):
    """Per (core, class): cluster rows sharing cols (capped union-find), then
    pack clusters into tiles of <=P slots / <=W rows with slot-granular
    splitting so tiles fill to exactly P.  A slot serves every clustered edge
    with that col (bval column gets several nonzeros).

    Returns list of tiles: (slots_cols, edge_slot, edge_row_global, edge_val)."""
    pres, inv_r, deg = np.unique(rj, return_inverse=True, return_counts=True)
    nr = len(pres)
    if nr == 0:
        return []
    parent = np.arange(nr)
    szr = np.ones(nr, np.int64)
    sze = deg.copy().astype(np.int64)

    def find(a):
        while parent[a] != a:
            parent[a] = parent[parent[a]]
            a = parent[a]
        return a

    order = np.argsort(cj, kind="stable")
    sc = cj[order]
    sr = inv_r[order]
    same = np.nonzero(sc[1:] == sc[:-1])[0]
    for i in same:
        a, b = find(sr[i]), find(sr[i + 1])
        if a == b:
            continue
        if szr[a] + szr[b] <= MAXR and sze[a] + sze[b] <= MAXE:
            parent[b] = a
            szr[a] += szr[b]
            sze[a] += sze[b]
    root = np.fromiter((find(i) for i in range(nr)), np.int64, nr)

    corder = np.argsort(root, kind="stable")
    croots = root[corder]
    starts = np.nonzero(np.concatenate([[True], croots[1:] != croots[:-1]]))[0]
    bounds = np.concatenate([starts, [nr]])

    er_order = np.argsort(inv_r, kind="stable")
    row_off = np.concatenate([[0], np.cumsum(deg)])
    items = []
    for gi in range(len(starts)):
        ranks = corder[bounds[gi] : bounds[gi + 1]]
        eidx = np.concatenate([er_order[row_off[r] : row_off[r + 1]] for r in ranks])
        ccols = cj[eidx]
        so = np.argsort(ccols, kind="stable")  # group edges by col
        scols = ccols[so]
        uniq_mask = np.concatenate([[True], scols[1:] != scols[:-1]])
        uniq = scols[uniq_mask]
        eoff = np.concatenate([np.nonzero(uniq_mask)[0], [len(scols)]])
        items.append(
            _Item(uniq, eoff, pres[inv_r[eidx[so]]], vj[eidx[so]], len(ranks))
        )

    # pack: by-size buckets, prefer largest whole item, split to fill
    maxd = max(len(it.cols) for it in items)
    by_size = [[] for _ in range(maxd + 1)]
    for it in items:
        by_size[len(it.cols)].append(it)
    navail = len(items)
    tiles = []
    while navail:
        cap = P
        roomw = W
        pieces = []
        cur = maxd
        while navail and cap > 0 and roomw > 0:
            while cur > 0 and not by_size[cur]:
                cur -= 1
            if cur == 0:
                break
            dd = min(cap, cur)
            while dd > 0 and not by_size[dd]:
                dd -= 1
            placed = False
            if dd > 0:
                cand = by_size[dd][-1]
                if cand.nrows <= roomw:
                    by_size[dd].pop()
                    navail -= 1
                    pieces.append(cand)
                    cap -= len(cand.cols)
                    roomw -= cand.nrows
                    placed = True
            if not placed:
                # split the largest remaining item to fill the tile
                it = by_size[cur].pop()
                piece, rem = _split_item(it, cap, roomw)
                if rem is not None:
                    by_size[len(rem.cols)].append(rem)
                    if len(rem.cols) > cur:
                        cur = len(rem.cols)
                else:
                    navail -= 1
                if piece is None:
                    break  # tile can't host even one slot of it
                pieces.append(piece)
                cap -= len(piece.cols)
                roomw -= piece.nrows
        if not pieces:
            break
        slots_cols = np.concatenate([p.cols for p in pieces])
        e_slot = np.concatenate(
            [
                np.repeat(np.arange(len(p.cols)) + so, np.diff(p.eoff))
                for p, so in zip(
                    pieces, np.cumsum([0] + [len(p.cols) for p in pieces])[:-1]
                )
            ]
        )
        e_row = np.concatenate([p.erow for p in pieces])
        e_val = np.concatenate([p.eval_ for p in pieces])
        tiles.append((slots_cols, e_slot, e_row, e_val))
    return tiles


def _prepare_core(rows, cols, vals, core):
    lo = core * RPC
    e0, e1 = np.searchsorted(rows, [lo, lo + RPC])
    r = (np.asarray(rows[e0:e1]) - lo).astype(np.int64)
    c = np.asarray(cols[e0:e1]).astype(np.int64)
    v = np.asarray(vals[e0:e1]).astype(np.float32)
    out = []
    for j in range(NCLS):
        m = c % NCLS == j
        out.append(_cluster_and_pack(r[m], c[m].astype(np.int32), v[m]))
    return out


def _nsb_for(per_core):
    need = max(sum(len(cl) for cl in pc) for pc in per_core)
    return (need + TPS - 1) // TPS


def _assemble_core(per_core_k, nsb):
    """Tiles fill program slots sequentially -- the matmul consumes every
    gathered row the same way, so class structure is host-side only."""
    nt = nsb * TPS
    idx_all = np.zeros((128, nt), np.int32)
    bval = np.zeros((128, nsb * 512), np.float32)
    prow_l, ppos_l = [], []
    all_tiles = [t for cl in per_core_k for t in cl]
    assert len(all_tiles) <= nt, (len(all_tiles), nt)
    for t, (scols, e_slot, e_row, e_val) in enumerate(all_tiles):
        s, tl = divmod(t, TPS)
        ns = len(scols)
        idx_all[:ns, t] = scols
        rws, e_rloc = np.unique(e_row, return_inverse=True)
        assert len(rws) <= W and ns <= P
        np.add.at(bval, (e_slot, s * 512 + tl * W + e_rloc), e_val)
        prow_l.append(rws)
        cc = tl * W + np.arange(len(rws))
        ppos_l.append((s * 128 + cc % 128) * 4 + cc // 128)
    prow = np.concatenate(prow_l) if prow_l else np.zeros(0, np.int64)
    ppos = np.concatenate(ppos_l) if ppos_l else np.zeros(0, np.int64)
    return idx_all, bval.astype(ml_dtypes.bfloat16), prow, ppos


def _build_program(nsb):
    nt = nsb * TPS
    nc = bacc.Bacc("TRN2", target_bir_lowering=False, debug=False)
    x = nc.dram_tensor("x", [N_NODES, F], f32, kind="ExternalInput")
    idx = nc.dram_tensor("idx", [128, nt], i32, kind="ExternalInput")
    bval = nc.dram_tensor("bval", [128, nsb * 512], bf16, kind="ExternalInput")
    w = nc.dram_tensor("w", [F, F], bf16, kind="ExternalInput")
    out = nc.dram_tensor("out", [nsb, 128, 128], f32, kind="ExternalOutput")

    with tile.TileContext(nc) as tc:
        with (
            tc.tile_pool(name="const", bufs=1) as cpool,
            tc.tile_pool(name="bv", bufs=5) as bvpool,
            tc.tile_pool(name="g", bufs=8) as gpool,
            tc.tile_pool(name="zt", bufs=5, space="PSUM") as ztpool,
            tc.tile_pool(name="po", bufs=2, space="PSUM") as popool,
            tc.tile_pool(name="ev", bufs=5) as evpool,
        ):
            wt = cpool.tile([F, F], bf16)
            nc.sync.dma_start(wt[:], w[:])
            # all gather offsets stay SBUF-resident: zero per-tile load stalls
            idxr = cpool.tile([128, nt], i32)
            nc.sync.dma_start(idxr[:], idx[:])
            for s in range(nsb):
                gt = gpool.tile([128, TPS * F], bf16, tag="g")
                for tl in range(TPS):
                    t = s * TPS + tl
                    nc.gpsimd.indirect_dma_start(
                        out=gt[:, tl * F : (tl + 1) * F],
                        out_offset=None,
                        in_=x[:],
                        in_offset=IndirectOffsetOnAxis(
                            ap=idxr[:, t : t + 1], axis=0
                        ),
                    )
                bvt = bvpool.tile([128, 512], bf16, tag="bv")
                nc.sync.dma_start(bvt[:], bval[:, s * 512 : (s + 1) * 512])
                zt = ztpool.tile([F, 512], f32, tag="zt")
                for tl in range(TPS):
                    nc.tensor.matmul(
                        out=zt[:, tl * W : (tl + 1) * W],
                        lhsT=gt[:, tl * F : (tl + 1) * F],
                        rhs=bvt[:, tl * W : (tl + 1) * W],
                        start=True,
                        stop=True,
                    )
                zsb = evpool.tile([F, 512], bf16, tag="zsb")
                nc.scalar.copy(zsb[:], zt[:])
                pot = popool.tile([128, 4 * F], f32, tag="po")
                for ci in range(4):
                    nc.tensor.matmul(
                        out=pot[:, ci * F : (ci + 1) * F],
                        lhsT=zsb[:, ci * 128 : (ci + 1) * 128],
                        rhs=wt[:],
                        start=True,
                        stop=True,
                    )
                ott = evpool.tile([128, 4 * F], f32, tag="ot")
                nc.vector.tensor_copy(ott[:], pot[:])
                nc.scalar.dma_start(out[s], ott[:])
    nc.compile()
    return nc


def prepare(x, rows, cols, vals, weight):
    """Host packing + compile.  Returns (nc, in_maps, maps, nsb)."""
    x = np.ascontiguousarray(np.asarray(x, dtype=np.float32))
    rows = np.asarray(rows)
    cols = np.asarray(cols)
    vals = np.asarray(vals, dtype=np.float32)
    weight = np.ascontiguousarray(np.asarray(weight, dtype=np.float32))

    per_core = [_prepare_core(rows, cols, vals, k) for k in range(N_CORES)]
    nsb = _nsb_for(per_core)

    if nsb not in _compiled_cache:
        _compiled_cache[nsb] = _build_program(nsb)
    nc = _compiled_cache[nsb]

    in_maps = []
    maps = []
    for k in range(N_CORES):
        idx_all, bval, prow, ppos = _assemble_core(per_core[k], nsb)
        maps.append((prow, ppos))
        in_maps.append(
            {"x": x, "idx": idx_all, "bval": bval, "w": weight.astype(ml_dtypes.bfloat16)}
        )
    return nc, in_maps, maps, nsb


def gather_output(res, maps, nsb):
    out_full = np.zeros((N_NODES, F), np.float32)
    for k in range(N_CORES):
        dev = np.asarray(res.results[k]["out"], dtype=np.float32)
        dev = dev.reshape(nsb * 128 * 4, 32)
        prow, ppos = maps[k]
        np.add.at(out_full, k * RPC + prow, dev[ppos])
    return out_full


def kernel(x, rows, cols, vals, weight):
    nc, in_maps, maps, nsb = prepare(x, rows, cols, vals, weight)
    res = run_bass_kernel_spmd(nc, in_maps, list(range(N_CORES)))
    return gather_output(res, maps, nsb)
